# revision 11
# baseline (speedup 1.0000x reference)
"""Multi-head attention (B=8, S=1024, D=1024, H=16) on 8 TRN2 NeuronCores.

Sharding: data-parallel over the batch dim - core b computes batch element b
end-to-end (projections + attention + output projection). No collectives.

Compute structure (all matmuls full-array 128-contraction, N=512, bf16):
trace analysis showed tiled matmuls (row_grp/col_grp) pay a serial ~95 ns
LDWEIGHTS on every matmul (no background-buffer hiding; walrus runs with
--enable-ldw-opt=false), while full-array matmuls issue at ~fill rate
(~225 ns for N=512). Fill cost is N cycles regardless of K/M, so the
padded-scores + 65-row-vaug structure is fill-optimal: 1024 matmuls/iter,
~220 us PE floor.

  - Q^T/K^T in [e, s] layout; K^T written into zero-padded kpadA (head 2et
    in rows 0:64) / kpadB (head 2et+1 in rows 64:128) so scores contract
    over all 128 partitions.
  - V (+bias) written into V_aug tiles [128, H, 65], last column 1.0: the
    ctx matmul computes the softmax denominator as psum row 64 for free.
  - exp on ScalarE straight out of PSUM ([128,1024] per (head, jt),
    ~147 us/iter total, hidden under PE work by the schedule).
  - normalize: ctx+denom PSUM rows copied to SBUF bf16 immediately (frees
    the 2 ctx banks for the next head), denominator row to f32,
    reciprocal_approx_fast, GpSimd partition_broadcast, DVE multiply.

Schedule: Q/K projections for e-chunk et+1 and the V projection are
emitted as 8-matmul half-chunk fillers INSIDE the attention jt-loop of
earlier heads, so ScalarE's exp stream overlaps PE work. Input DMAs are
emitted in consumption order ((wq,xq) pairs first) so iteration i+1's
lead-in projection is not blocked behind unrelated DMAs. Output
projection at the tail; the For_i repeat loop overlaps iteration i+1's
lead-in with iteration i's tail.

PSUM (8 banks): scores [128,1024] bufs=2 -> 4, proj [128,512] bufs=2 -> 2,
ctx chains [65,512] bufs=2 -> 2.
"""

import numpy as np
import ml_dtypes

import concourse.bass as bass
import concourse.mybir as mybir
import concourse.tile as tile
from concourse import bacc
from concourse.bass_utils import run_bass_kernel_spmd

BF = ml_dtypes.bfloat16

B, S, D, H = 8, 1024, 1024, 16
DK = D // H            # 64
P = 128
KT = D // P            # 8 contraction chunks
ET = D // P            # 8 e-tiles
ST = S // P            # 8 s/j tiles
FREE = 512             # one PSUM bank of fp32
NIH = S // FREE        # 2 i-halves
N_CORES = 8

F32 = mybir.dt.float32
BF16 = mybir.dt.bfloat16
ADD = mybir.AluOpType.add
MULT = mybir.AluOpType.mult
EXP = mybir.ActivationFunctionType.Exp
SCALE = float(1.0 / np.sqrt(DK))


def build_nc(repeat: int = 1):
    """Build + compile the SPMD single-core program (same NEFF on all cores)."""
    nc = bacc.Bacc("TRN2", target_bir_lowering=False, debug=False,
                   num_devices=N_CORES)

    xq_d = nc.dram_tensor("xq_t", [D, S], BF16, kind="ExternalInput")
    xk_d = nc.dram_tensor("xk_t", [D, S], BF16, kind="ExternalInput")
    xv_d = nc.dram_tensor("xv_t", [D, S], BF16, kind="ExternalInput")
    wq_d = nc.dram_tensor("wq_t", [D, D], BF16, kind="ExternalInput")
    wk_d = nc.dram_tensor("wk_t", [D, D], BF16, kind="ExternalInput")
    wv_d = nc.dram_tensor("wv_t", [D, D], BF16, kind="ExternalInput")
    wo_d = nc.dram_tensor("wo_t", [D, D], BF16, kind="ExternalInput")
    bq_d = nc.dram_tensor("bq_r", [P, ET], F32, kind="ExternalInput")
    bk_d = nc.dram_tensor("bk_r", [P, ET], F32, kind="ExternalInput")
    bvb_d = nc.dram_tensor("bvb", [P, D], BF16, kind="ExternalInput")
    bob_d = nc.dram_tensor("bob", [P, D], BF16, kind="ExternalInput")
    out_d = nc.dram_tensor("out", [S, D], F32, kind="ExternalOutput")

    with tile.TileContext(nc) as tc:
        with tc.tile_pool(name="xin", bufs=3 * KT) as xin, \
             tc.tile_pool(name="wgt", bufs=KT) as wgt, \
             tc.tile_pool(name="wqk", bufs=2 * KT) as wqkp, \
             tc.tile_pool(name="qk", bufs=ET) as qkp, \
             tc.tile_pool(name="kpd", bufs=2 * ET) as kpd, \
             tc.tile_pool(name="vau", bufs=ST) as vau, \
             tc.tile_pool(name="att", bufs=3) as att, \
             tc.tile_pool(name="ctx", bufs=ET) as ctxp, \
             tc.tile_pool(name="csb", bufs=2) as csbp, \
             tc.tile_pool(name="outp", bufs=1) as outp, \
             tc.tile_pool(name="d2p", bufs=1) as d2p, \
             tc.tile_pool(name="rcp2", bufs=1) as rcp2, \
             tc.tile_pool(name="rbp", bufs=1) as rbp, \
             tc.tile_pool(name="cst", bufs=1) as cst, \
             tc.tile_pool(name="sc", bufs=2, space="PSUM") as scp, \
             tc.tile_pool(name="pj", bufs=2, space="PSUM") as pjp, \
             tc.tile_pool(name="cps", bufs=2, space="PSUM") as cpsp:

            # ---- constants (outside the repeat loop) ----
            bq_sb = cst.tile([P, ET], F32, name="bq_sb")
            bk_sb = cst.tile([P, ET], F32, name="bk_sb")
            bvb_sb = cst.tile([P, D], BF16, name="bvb_sb")
            bob_sb = cst.tile([P, D], BF16, name="bob_sb")
            nc.sync.dma_start(out=bq_sb[:], in_=bq_d[:])
            nc.sync.dma_start(out=bk_sb[:], in_=bk_d[:])
            nc.sync.dma_start(out=bvb_sb[:], in_=bvb_d[:])
            nc.sync.dma_start(out=bob_sb[:], in_=bob_d[:])

            # resident Q/K projection weights (loop-invariant)
            wq_sb, wk_sb = [], []
            for k in range(KT):
                t = wqkp.tile([P, D], BF16, tag="wqk", name=f"wq{k}")
                nc.sync.dma_start(out=t[:], in_=wq_d[k * P:(k + 1) * P, :])
                wq_sb.append(t)
            for k in range(KT):
                t = wqkp.tile([P, D], BF16, tag="wqk", name=f"wk{k}")
                nc.sync.dma_start(out=t[:], in_=wk_d[k * P:(k + 1) * P, :])
                wk_sb.append(t)

            # zero-padded K^T tiles: kpadA holds head 2et in rows 0:64,
            # kpadB holds head 2et+1 in rows 64:128; other halves stay 0.
            kpadA = [kpd.tile([P, S], BF16, tag="kpd", name=f"kpdA{et}")
                     for et in range(ET)]
            kpadB = [kpd.tile([P, S], BF16, tag="kpd", name=f"kpdB{et}")
                     for et in range(ET)]
            for et in range(ET):
                nc.vector.memset(kpadA[et][64:P, :], 0.0)
                nc.vector.memset(kpadB[et][0:64, :], 0.0)

            # V_aug [128, H, 65]: last column 1.0 (denominator trick)
            vaug = [vau.tile([P, H, DK + 1], BF16, tag="vaug",
                             name=f"vaug{st}") for st in range(ST)]
            for st in range(ST):
                nc.vector.memset(vaug[st][:, :, DK:DK + 1], 1.0)

            def body():
                # ---------- input DMA emission, in consumption order ------
                xq_sb, xk_sb = [], []
                for k in range(KT):
                    t = xin.tile([P, S], BF16, tag="x", name=f"xq{k}")
                    nc.sync.dma_start(out=t[:], in_=xq_d[k * P:(k + 1) * P, :])
                    xq_sb.append(t)
                for k in range(KT):
                    t = xin.tile([P, S], BF16, tag="x", name=f"xk{k}")
                    nc.sync.dma_start(out=t[:], in_=xk_d[k * P:(k + 1) * P, :])
                    xk_sb.append(t)
                xv_sb, wv_sb, wo_sb = [], [], []

                def dma_xv():
                    for k in range(KT):
                        t = xin.tile([P, S], BF16, tag="x", name=f"xv{k}")
                        nc.sync.dma_start(out=t[:],
                                          in_=xv_d[k * P:(k + 1) * P, :])
                        xv_sb.append(t)
                        t = wgt.tile([P, D], BF16, tag="w", name=f"wv{k}")
                        nc.sync.dma_start(out=t[:],
                                          in_=wv_d[k * P:(k + 1) * P, :])
                        wv_sb.append(t)

                def dma_wo():
                    for k in range(KT):
                        t = wgt.tile([P, D], BF16, tag="w", name=f"wo{k}")
                        nc.sync.dma_start(out=t[:],
                                          in_=wo_d[k * P:(k + 1) * P, :])
                        wo_sb.append(t)

                qt_sb = [None] * ET   # Q^T [e, s] bf16
                ctxt_sb = [ctxp.tile([P, S], BF16, tag="ctx",
                                     name=f"ctxt{et}") for et in range(ET)]

                # ---------- projection fillers ----------
                # each subunit = one 8-matmul accumulation chain into one
                # PSUM bank + its bias eviction (DVE overlaps the next
                # subunit's matmuls via the 2-buffer pj pool).
                def qk_half(nm, et, ih):
                    def go():
                        x_sb = xq_sb if nm == "q" else xk_sb
                        w_sb = wq_sb if nm == "q" else wk_sb
                        ps = pjp.tile([P, FREE], F32, tag="pj",
                                      name=f"{nm}ps{et}_{ih}")
                        sl = slice(ih * FREE, (ih + 1) * FREE)
                        for k in range(KT):
                            nc.tensor.matmul(
                                ps[:],
                                w_sb[k][:, et * P:(et + 1) * P],
                                x_sb[k][:, sl],
                                start=(k == 0), stop=(k == KT - 1))
                        if nm == "q":
                            if qt_sb[et] is None:
                                qt_sb[et] = qkp.tile([P, S], BF16, tag="qk",
                                                     name=f"qt{et}")
                            nc.vector.tensor_scalar(
                                out=qt_sb[et][:, sl], in0=ps[:],
                                scalar1=bq_sb[:, et:et + 1],
                                scalar2=None, op0=ADD)
                        else:
                            nc.vector.tensor_scalar(
                                out=kpadA[et][0:64, sl], in0=ps[0:64, :],
                                scalar1=bk_sb[0:64, et:et + 1],
                                scalar2=None, op0=ADD)
                            nc.vector.tensor_scalar(
                                out=kpadB[et][64:P, sl], in0=ps[64:P, :],
                                scalar1=bk_sb[64:P, et:et + 1],
                                scalar2=None, op0=ADD)
                    return go

                def v_half(st, eh):
                    def go():
                        ps = pjp.tile([P, FREE], F32, tag="pj",
                                      name=f"vps{st}_{eh}")
                        sl = slice(eh * FREE, (eh + 1) * FREE)
                        for k in range(KT):
                            nc.tensor.matmul(
                                ps[:],
                                xv_sb[k][:, st * P:(st + 1) * P],
                                wv_sb[k][:, sl],
                                start=(k == 0), stop=(k == KT - 1))
                        nc.vector.tensor_tensor(
                            out=vaug[st][:, eh * ET:(eh + 1) * ET, 0:DK],
                            in0=ps[:].rearrange("p (h c) -> p h c", h=ET),
                            in1=bvb_sb[:, sl].rearrange(
                                "p (h c) -> p h c", h=ET),
                            op=ADD)
                    return go

                def o_half(st, eh):
                    ps = pjp.tile([P, FREE], F32, tag="pj",
                                  name=f"ops{st}_{eh}")
                    sl = slice(eh * FREE, (eh + 1) * FREE)
                    for k in range(KT):
                        nc.tensor.matmul(
                            ps[:],
                            ctxt_sb[k][:, st * P:(st + 1) * P],
                            wo_sb[k][:, sl],
                            start=(k == 0), stop=(k == KT - 1))
                    return ps

                # per-head filler schedules (list of thunks per jt step)
                def filler_schedule(h):
                    units = []
                    if h == 0:
                        for st in range(2, ST):
                            units.append(v_half(st, 0))
                            units.append(v_half(st, 1))
                    elif h == 1:
                        for ih in range(NIH):
                            units.append(qk_half("q", 1, ih))
                        for ih in range(NIH):
                            units.append(qk_half("k", 1, ih))
                    elif h <= 13:
                        et = h // 2 + 1
                        nm = "q" if h % 2 == 0 else "k"
                        for ih in range(NIH):
                            units.append(qk_half(nm, et, ih))
                        if h == 12:
                            units.append(dma_wo)
                    steps = [[] for _ in range(ST)]
                    for i, u in enumerate(units):
                        steps[(i * ST) // max(len(units), 1) % ST].append(u)
                    return steps

                # ---------- attention ----------
                def scores_exp(h, jt):
                    et, half = h // 2, h % 2
                    kp = kpadA[et] if half == 0 else kpadB[et]
                    sc = scp.tile([P, S], F32, tag="sc", name=f"sc{h}_{jt}")
                    jsl = slice(jt * P, (jt + 1) * P)
                    for ih in range(NIH):
                        nc.tensor.matmul(
                            sc[:, ih * FREE:(ih + 1) * FREE],
                            kp[:, jsl],
                            qt_sb[et][:, ih * FREE:(ih + 1) * FREE],
                            start=True, stop=True)
                    a = att.tile([P, S], BF16, tag="attn", name=f"at{h}_{jt}")
                    nc.scalar.activation(a[:], sc[:], EXP, scale=SCALE)
                    return a

                def ctx_jt(h, jt, cps, a):
                    first, last = (jt == 0), (jt == ST - 1)
                    for ih in range(NIH):
                        nc.tensor.matmul(
                            cps[ih][:],
                            vaug[jt][:, h, :],
                            a[:, ih * FREE:(ih + 1) * FREE],
                            start=first, stop=last)

                def normalize(h, cps):
                    et, half = h // 2, h % 2
                    # evict ctx+denominator to SBUF bf16 (frees cps banks)
                    cs = csbp.tile([DK + 1, S], BF16, tag="cs",
                                   name=f"cs{h}")
                    for ih in range(NIH):
                        nc.vector.tensor_copy(
                            out=cs[:, ih * FREE:(ih + 1) * FREE],
                            in_=cps[ih][:])
                    d2 = d2p.tile([1, S], F32, tag="d2", name=f"d2_{h}")
                    nc.vector.tensor_copy(out=d2[:], in_=cs[DK:DK + 1, :])
                    rcp = rcp2.tile([1, S], F32, tag="rcp", name=f"rcp{h}")
                    nc.vector.reciprocal_approx_fast(out=rcp[:], in_=d2[:])
                    rb = rbp.tile([DK, S], F32, tag="rb", name=f"rb{h}")
                    nc.gpsimd.partition_broadcast(rb[:], rcp[0:1, :])
                    pr = slice(half * DK, (half + 1) * DK)
                    nc.vector.tensor_tensor(
                        out=ctxt_sb[et][pr, :], in0=cs[0:DK, :],
                        in1=rb[:], op=MULT)

                # ---------- emission schedule ----------
                for ih in range(NIH):
                    qk_half("q", 0, ih)()
                for ih in range(NIH):
                    qk_half("k", 0, ih)()
                dma_xv()
                for st in range(2):
                    for eh in range(NIH):
                        v_half(st, eh)()

                for h in range(H):
                    steps = filler_schedule(h)
                    cps = [cpsp.tile([DK + 1, FREE], F32, tag="cps",
                                     name=f"cps{h}_{ih}") for ih in range(NIH)]
                    prev_a = None
                    for jt in range(ST):
                        a = scores_exp(h, jt)
                        for u in steps[jt][:len(steps[jt]) // 2]:
                            u()
                        if prev_a is not None:
                            ctx_jt(h, jt - 1, cps, prev_a)
                        for u in steps[jt][len(steps[jt]) // 2:]:
                            u()
                        prev_a = a
                    ctx_jt(h, ST - 1, cps, prev_a)
                    normalize(h, cps)

                # ---------- output projection ----------
                for st in range(ST):
                    ps = [o_half(st, eh) for eh in range(NIH)]
                    o = outp.tile([P, D], F32, tag="o", name=f"o{st}")
                    for eh in range(NIH):
                        sl = slice(eh * FREE, (eh + 1) * FREE)
                        nc.vector.tensor_tensor(out=o[:, sl], in0=ps[eh][:],
                                                in1=bob_sb[:, sl], op=ADD)
                    nc.sync.dma_start(out=out_d[st * P:(st + 1) * P, :],
                                      in_=o[:])

            if repeat == 1:
                body()
            else:
                with tc.For_i(0, repeat, 1, staggered_reset=True) as _:
                    body()

    nc.compile()
    return nc


_NC_CACHE: dict = {}


def get_nc(repeat: int = 1):
    if repeat not in _NC_CACHE:
        _NC_CACHE[repeat] = build_nc(repeat)
    return _NC_CACHE[repeat]


def make_in_maps(query, key_, value, w_q, b_q, w_k, b_k, w_v, b_v, w_o, b_o):
    shared = {
        "wq_t": np.ascontiguousarray(np.asarray(w_q, np.float32).T).astype(BF),
        "wk_t": np.ascontiguousarray(np.asarray(w_k, np.float32).T).astype(BF),
        "wv_t": np.ascontiguousarray(np.asarray(w_v, np.float32).T).astype(BF),
        "wo_t": np.ascontiguousarray(np.asarray(w_o, np.float32).T).astype(BF),
        "bq_r": np.ascontiguousarray(
            np.asarray(b_q, np.float32).reshape(ET, P).T),
        "bk_r": np.ascontiguousarray(
            np.asarray(b_k, np.float32).reshape(ET, P).T),
        "bvb": np.ascontiguousarray(
            np.tile(np.asarray(b_v, np.float32)[None, :], (P, 1))).astype(BF),
        "bob": np.ascontiguousarray(
            np.tile(np.asarray(b_o, np.float32)[None, :], (P, 1))).astype(BF),
    }
    q = np.asarray(query, np.float32)
    k = np.asarray(key_, np.float32)
    v = np.asarray(value, np.float32)
    in_maps = []
    for b in range(B):
        m = dict(shared)
        m["xq_t"] = np.ascontiguousarray(q[b].T).astype(BF)
        m["xk_t"] = np.ascontiguousarray(k[b].T).astype(BF)
        m["xv_t"] = np.ascontiguousarray(v[b].T).astype(BF)
        in_maps.append(m)
    return in_maps


def run(in_maps, repeat: int = 1):
    nc = get_nc(repeat)
    res = run_bass_kernel_spmd(nc, in_maps, list(range(N_CORES)))
    return np.stack([np.asarray(res.results[i]["out"], np.float32)
                     for i in range(B)])


def kernel(query, key_, value, w_q, b_q, w_k, b_k, w_v, b_v, w_o, b_o):
    in_maps = make_in_maps(query, key_, value, w_q, b_q, w_k, b_k,
                           w_v, b_v, w_o, b_o)
    return run(in_maps, repeat=1)


if __name__ == "__main__":
    rng = np.random.default_rng(0)
    sc = 1.0 / np.sqrt(D)
    inputs = dict(
        query=rng.standard_normal((B, S, D), dtype=np.float32),
        key_=rng.standard_normal((B, S, D), dtype=np.float32),
        value=rng.standard_normal((B, S, D), dtype=np.float32),
        w_q=rng.standard_normal((D, D), dtype=np.float32) * sc,
        b_q=np.zeros(D, np.float32),
        w_k=rng.standard_normal((D, D), dtype=np.float32) * sc,
        b_k=np.zeros(D, np.float32),
        w_v=rng.standard_normal((D, D), dtype=np.float32) * sc,
        b_v=np.zeros(D, np.float32),
        w_o=rng.standard_normal((D, D), dtype=np.float32) * sc,
        b_o=np.zeros(D, np.float32),
    )
    out = kernel(**inputs)
    print("out", out.shape, out.dtype, float(np.abs(out).max()))


# revision 12
# speedup vs baseline: 1.1175x; 1.1175x over previous
"""Multi-head attention (B=8, S=1024, D=1024, H=16) on 8 TRN2 NeuronCores.

Sharding: data-parallel over the batch dim - core b computes batch element b
end-to-end (projections + attention + output projection). No collectives.

Compute structure (all matmuls full-array 128-contraction, N=512, bf16):
trace analysis showed tiled matmuls (row_grp/col_grp) pay a serial ~95 ns
LDWEIGHTS on every matmul (no background-buffer hiding; walrus runs with
--enable-ldw-opt=false), while full-array matmuls issue at ~fill rate
(~225 ns for N=512). Fill cost is N cycles regardless of K/M, so the
padded-scores + 65-row-vaug structure is fill-optimal: 1024 matmuls/iter,
~220 us PE floor.

  - Q^T/K^T in [e, s] layout; K^T written into zero-padded kpadA (head 2et
    in rows 0:64) / kpadB (head 2et+1 in rows 64:128) so scores contract
    over all 128 partitions.
  - V (+bias) written into V_aug tiles [128, H, 65], last column 1.0: the
    ctx matmul computes the softmax denominator as psum row 64 for free.
  - exp on ScalarE straight out of PSUM ([128,1024] per (head, jt),
    ~147 us/iter total, hidden under PE work by the schedule).
  - normalize: ctx+denom PSUM rows copied to SBUF bf16 immediately (frees
    the 2 ctx banks for the next head), denominator row to f32,
    reciprocal_approx_fast, GpSimd partition_broadcast, DVE multiply.

Schedule: Q/K projections for e-chunk et+1 and the V projection are
emitted as 8-matmul half-chunk fillers INSIDE the attention jt-loop of
earlier heads, so ScalarE's exp stream overlaps PE work. Input DMAs are
emitted in consumption order ((wq,xq) pairs first) so iteration i+1's
lead-in projection is not blocked behind unrelated DMAs. Output
projection at the tail; the For_i repeat loop overlaps iteration i+1's
lead-in with iteration i's tail.

PSUM (8 banks): scores [128,1024] bufs=2 -> 4, proj [128,512] bufs=2 -> 2,
ctx chains [65,512] bufs=2 -> 2.
"""

import numpy as np
import ml_dtypes

import concourse.bass as bass
import concourse.mybir as mybir
import concourse.tile as tile
from concourse import bacc
from concourse.bass_utils import run_bass_kernel_spmd

BF = ml_dtypes.bfloat16

B, S, D, H = 8, 1024, 1024, 16
DK = D // H            # 64
P = 128
KT = D // P            # 8 contraction chunks
ET = D // P            # 8 e-tiles
ST = S // P            # 8 s/j tiles
FREE = 512             # one PSUM bank of fp32
NIH = S // FREE        # 2 i-halves
N_CORES = 8

F32 = mybir.dt.float32
BF16 = mybir.dt.bfloat16
ADD = mybir.AluOpType.add
MULT = mybir.AluOpType.mult
EXP = mybir.ActivationFunctionType.Exp
SCALE = float(1.0 / np.sqrt(DK))


def build_nc(repeat: int = 1):
    """Build + compile the SPMD single-core program (same NEFF on all cores)."""
    nc = bacc.Bacc("TRN2", target_bir_lowering=False, debug=False,
                   num_devices=N_CORES)

    xq_d = nc.dram_tensor("xq_t", [D, S], BF16, kind="ExternalInput")
    xk_d = nc.dram_tensor("xk_t", [D, S], BF16, kind="ExternalInput")
    xv_d = nc.dram_tensor("xv_t", [D, S], BF16, kind="ExternalInput")
    wq_d = nc.dram_tensor("wq_t", [D, D], BF16, kind="ExternalInput")
    wk_d = nc.dram_tensor("wk_t", [D, D], BF16, kind="ExternalInput")
    wv_d = nc.dram_tensor("wv_t", [D, D], BF16, kind="ExternalInput")
    wo_d = nc.dram_tensor("wo_t", [D, D], BF16, kind="ExternalInput")
    bq_d = nc.dram_tensor("bq_r", [P, ET], F32, kind="ExternalInput")
    bk_d = nc.dram_tensor("bk_r", [P, ET], F32, kind="ExternalInput")
    bvb_d = nc.dram_tensor("bvb", [P, D], BF16, kind="ExternalInput")
    bob_d = nc.dram_tensor("bob", [P, D], BF16, kind="ExternalInput")
    out_d = nc.dram_tensor("out", [S, D], F32, kind="ExternalOutput")

    with tile.TileContext(nc) as tc:
        with tc.tile_pool(name="xin", bufs=3 * KT) as xin, \
             tc.tile_pool(name="wgt", bufs=KT) as wgt, \
             tc.tile_pool(name="wqk", bufs=2 * KT) as wqkp, \
             tc.tile_pool(name="qk", bufs=ET) as qkp, \
             tc.tile_pool(name="kpd", bufs=2 * ET) as kpd, \
             tc.tile_pool(name="vau", bufs=ST) as vau, \
             tc.tile_pool(name="att", bufs=3) as att, \
             tc.tile_pool(name="ctx", bufs=ET) as ctxp, \
             tc.tile_pool(name="csb", bufs=2) as csbp, \
             tc.tile_pool(name="outp", bufs=1) as outp, \
             tc.tile_pool(name="d2p", bufs=1) as d2p, \
             tc.tile_pool(name="rcp2", bufs=1) as rcp2, \
             tc.tile_pool(name="rbp", bufs=1) as rbp, \
             tc.tile_pool(name="cst", bufs=1) as cst, \
             tc.tile_pool(name="sc", bufs=2, space="PSUM") as scp, \
             tc.tile_pool(name="pj", bufs=2, space="PSUM") as pjp, \
             tc.tile_pool(name="cps", bufs=2, space="PSUM") as cpsp:

            # ---- constants (outside the repeat loop) ----
            bq_sb = cst.tile([P, ET], F32, name="bq_sb")
            bk_sb = cst.tile([P, ET], F32, name="bk_sb")
            bvb_sb = cst.tile([P, D], BF16, name="bvb_sb")
            bob_sb = cst.tile([P, D], BF16, name="bob_sb")
            nc.sync.dma_start(out=bq_sb[:], in_=bq_d[:])
            nc.sync.dma_start(out=bk_sb[:], in_=bk_d[:])
            nc.sync.dma_start(out=bvb_sb[:], in_=bvb_d[:])
            nc.sync.dma_start(out=bob_sb[:], in_=bob_d[:])

            # resident Q/K projection weights (loop-invariant)
            wq_sb, wk_sb = [], []
            for k in range(KT):
                t = wqkp.tile([P, D], BF16, tag="wqk", name=f"wq{k}")
                nc.sync.dma_start(out=t[:], in_=wq_d[k * P:(k + 1) * P, :])
                wq_sb.append(t)
            for k in range(KT):
                t = wqkp.tile([P, D], BF16, tag="wqk", name=f"wk{k}")
                nc.sync.dma_start(out=t[:], in_=wk_d[k * P:(k + 1) * P, :])
                wk_sb.append(t)

            # zero-padded K^T tiles: kpadA holds head 2et in rows 0:64,
            # kpadB holds head 2et+1 in rows 64:128; other halves stay 0.
            kpadA = [kpd.tile([P, S], BF16, tag="kpd", name=f"kpdA{et}")
                     for et in range(ET)]
            kpadB = [kpd.tile([P, S], BF16, tag="kpd", name=f"kpdB{et}")
                     for et in range(ET)]
            for et in range(ET):
                nc.vector.memset(kpadA[et][64:P, :], 0.0)
                nc.vector.memset(kpadB[et][0:64, :], 0.0)

            # V_aug [128, H, 65]: last column 1.0 (denominator trick)
            vaug = [vau.tile([P, H, DK + 1], BF16, tag="vaug",
                             name=f"vaug{st}") for st in range(ST)]
            for st in range(ST):
                nc.vector.memset(vaug[st][:, :, DK:DK + 1], 1.0)

            def body():
                # ---------- input DMA emission, in consumption order ------
                xq_sb, xk_sb = [], []
                for k in range(KT):
                    t = xin.tile([P, S], BF16, tag="x", name=f"xq{k}")
                    nc.sync.dma_start(out=t[:], in_=xq_d[k * P:(k + 1) * P, :])
                    xq_sb.append(t)
                for k in range(KT):
                    t = xin.tile([P, S], BF16, tag="x", name=f"xk{k}")
                    nc.sync.dma_start(out=t[:], in_=xk_d[k * P:(k + 1) * P, :])
                    xk_sb.append(t)
                xv_sb, wv_sb, wo_sb = [], [], []

                def dma_xv():
                    for k in range(KT):
                        t = xin.tile([P, S], BF16, tag="x", name=f"xv{k}")
                        nc.sync.dma_start(out=t[:],
                                          in_=xv_d[k * P:(k + 1) * P, :])
                        xv_sb.append(t)
                        t = wgt.tile([P, D], BF16, tag="w", name=f"wv{k}")
                        nc.sync.dma_start(out=t[:],
                                          in_=wv_d[k * P:(k + 1) * P, :])
                        wv_sb.append(t)

                def dma_wo():
                    for k in range(KT):
                        t = wgt.tile([P, D], BF16, tag="w", name=f"wo{k}")
                        nc.sync.dma_start(out=t[:],
                                          in_=wo_d[k * P:(k + 1) * P, :])
                        wo_sb.append(t)

                qt_sb = [None] * ET   # Q^T [e, s] bf16
                ctxt_sb = [ctxp.tile([P, S], BF16, tag="ctx",
                                     name=f"ctxt{et}") for et in range(ET)]

                # ---------- projection fillers ----------
                # each subunit = one 8-matmul accumulation chain into one
                # PSUM bank + its bias eviction (DVE overlaps the next
                # subunit's matmuls via the 2-buffer pj pool).
                def qk_half(nm, et, ih):
                    def go():
                        x_sb = xq_sb if nm == "q" else xk_sb
                        w_sb = wq_sb if nm == "q" else wk_sb
                        ps = pjp.tile([P, FREE], F32, tag="pj",
                                      name=f"{nm}ps{et}_{ih}")
                        sl = slice(ih * FREE, (ih + 1) * FREE)
                        for k in range(KT):
                            nc.tensor.matmul(
                                ps[:],
                                w_sb[k][:, et * P:(et + 1) * P],
                                x_sb[k][:, sl],
                                start=(k == 0), stop=(k == KT - 1))
                        if nm == "q":
                            if qt_sb[et] is None:
                                qt_sb[et] = qkp.tile([P, S], BF16, tag="qk",
                                                     name=f"qt{et}")
                            nc.vector.tensor_scalar(
                                out=qt_sb[et][:, sl], in0=ps[:],
                                scalar1=bq_sb[:, et:et + 1],
                                scalar2=None, op0=ADD)
                        else:
                            nc.vector.tensor_scalar(
                                out=kpadA[et][0:64, sl], in0=ps[0:64, :],
                                scalar1=bk_sb[0:64, et:et + 1],
                                scalar2=None, op0=ADD)
                            nc.vector.tensor_scalar(
                                out=kpadB[et][64:P, sl], in0=ps[64:P, :],
                                scalar1=bk_sb[64:P, et:et + 1],
                                scalar2=None, op0=ADD)
                    return go

                def v_half(st, eh):
                    def go():
                        ps = pjp.tile([P, FREE], F32, tag="pj",
                                      name=f"vps{st}_{eh}")
                        sl = slice(eh * FREE, (eh + 1) * FREE)
                        for k in range(KT):
                            nc.tensor.matmul(
                                ps[:],
                                xv_sb[k][:, st * P:(st + 1) * P],
                                wv_sb[k][:, sl],
                                start=(k == 0), stop=(k == KT - 1))
                        nc.vector.tensor_tensor(
                            out=vaug[st][:, eh * ET:(eh + 1) * ET, 0:DK],
                            in0=ps[:].rearrange("p (h c) -> p h c", h=ET),
                            in1=bvb_sb[:, sl].rearrange(
                                "p (h c) -> p h c", h=ET),
                            op=ADD)
                    return go

                def o_half(st, eh):
                    ps = pjp.tile([P, FREE], F32, tag="pj",
                                  name=f"ops{st}_{eh}")
                    sl = slice(eh * FREE, (eh + 1) * FREE)
                    for k in range(KT):
                        nc.tensor.matmul(
                            ps[:],
                            ctxt_sb[k][:, st * P:(st + 1) * P],
                            wo_sb[k][:, sl],
                            start=(k == 0), stop=(k == KT - 1))
                    return ps

                # per-head filler schedules (list of thunks per jt step).
                # QK(2..7) subunits are spread uniformly over heads 0-13 so
                # each projection lands >=2 heads before its consumer and
                # the DVE bias eviction is never on the scores critical
                # path; V fills head 0.
                qk_units = []
                for et in range(2, ET):
                    for nm in ("q", "k"):
                        for ih in range(NIH):
                            qk_units.append(qk_half(nm, et, ih))

                def filler_schedule(h):
                    units = []
                    if h == 0:
                        for st in range(2, ST):
                            units.append(v_half(st, 0))
                            units.append(v_half(st, 1))
                    if h <= 13:
                        n = len(qk_units)
                        units.extend(
                            qk_units[(h * n) // 14:((h + 1) * n) // 14])
                        if h == 11:
                            units.append(dma_wo)
                    steps = [[] for _ in range(ST)]
                    for i, u in enumerate(units):
                        steps[(i * ST) // max(len(units), 1) % ST].append(u)
                    return steps

                # ---------- attention ----------
                def scores_exp(h, jt):
                    et, half = h // 2, h % 2
                    kp = kpadA[et] if half == 0 else kpadB[et]
                    sc = scp.tile([P, S], F32, tag="sc", name=f"sc{h}_{jt}")
                    jsl = slice(jt * P, (jt + 1) * P)
                    for ih in range(NIH):
                        nc.tensor.matmul(
                            sc[:, ih * FREE:(ih + 1) * FREE],
                            kp[:, jsl],
                            qt_sb[et][:, ih * FREE:(ih + 1) * FREE],
                            start=True, stop=True)
                    a = att.tile([P, S], BF16, tag="attn", name=f"at{h}_{jt}")
                    nc.scalar.activation(a[:], sc[:], EXP, scale=SCALE)
                    return a

                def ctx_jt(h, jt, cps, a):
                    first, last = (jt == 0), (jt == ST - 1)
                    for ih in range(NIH):
                        nc.tensor.matmul(
                            cps[ih][:],
                            vaug[jt][:, h, :],
                            a[:, ih * FREE:(ih + 1) * FREE],
                            start=first, stop=last)

                def normalize(h, cps):
                    et, half = h // 2, h % 2
                    # evict ctx+denominator to SBUF bf16 (frees cps banks)
                    cs = csbp.tile([DK + 1, S], BF16, tag="cs",
                                   name=f"cs{h}")
                    for ih in range(NIH):
                        nc.vector.tensor_copy(
                            out=cs[:, ih * FREE:(ih + 1) * FREE],
                            in_=cps[ih][:])
                    d2 = d2p.tile([1, S], F32, tag="d2", name=f"d2_{h}")
                    nc.vector.tensor_copy(out=d2[:], in_=cs[DK:DK + 1, :])
                    rcp = rcp2.tile([1, S], F32, tag="rcp", name=f"rcp{h}")
                    nc.vector.reciprocal_approx_fast(out=rcp[:], in_=d2[:])
                    rb = rbp.tile([DK, S], F32, tag="rb", name=f"rb{h}")
                    nc.gpsimd.partition_broadcast(rb[:], rcp[0:1, :])
                    pr = slice(half * DK, (half + 1) * DK)
                    nc.vector.tensor_tensor(
                        out=ctxt_sb[et][pr, :], in0=cs[0:DK, :],
                        in1=rb[:], op=MULT)

                # ---------- emission schedule ----------
                for et in range(2):
                    for nm in ("q", "k"):
                        for ih in range(NIH):
                            qk_half(nm, et, ih)()
                    if et == 0:
                        dma_xv()
                for st in range(2):
                    for eh in range(NIH):
                        v_half(st, eh)()

                for h in range(H):
                    steps = filler_schedule(h)
                    cps = [cpsp.tile([DK + 1, FREE], F32, tag="cps",
                                     name=f"cps{h}_{ih}") for ih in range(NIH)]
                    prev_a = None
                    for jt in range(ST):
                        a = scores_exp(h, jt)
                        for u in steps[jt][:len(steps[jt]) // 2]:
                            u()
                        if prev_a is not None:
                            ctx_jt(h, jt - 1, cps, prev_a)
                        for u in steps[jt][len(steps[jt]) // 2:]:
                            u()
                        prev_a = a
                    ctx_jt(h, ST - 1, cps, prev_a)
                    normalize(h, cps)

                # ---------- output projection ----------
                for st in range(ST):
                    ps = [o_half(st, eh) for eh in range(NIH)]
                    o = outp.tile([P, D], F32, tag="o", name=f"o{st}")
                    for eh in range(NIH):
                        sl = slice(eh * FREE, (eh + 1) * FREE)
                        nc.vector.tensor_tensor(out=o[:, sl], in0=ps[eh][:],
                                                in1=bob_sb[:, sl], op=ADD)
                    nc.scalar.dma_start(out=out_d[st * P:(st + 1) * P, :],
                                        in_=o[:])

            if repeat == 1:
                body()
            else:
                with tc.For_i(0, repeat, 1, staggered_reset=True) as _:
                    body()

    nc.compile()
    return nc


_NC_CACHE: dict = {}


def get_nc(repeat: int = 1):
    if repeat not in _NC_CACHE:
        _NC_CACHE[repeat] = build_nc(repeat)
    return _NC_CACHE[repeat]


def make_in_maps(query, key_, value, w_q, b_q, w_k, b_k, w_v, b_v, w_o, b_o):
    shared = {
        "wq_t": np.ascontiguousarray(np.asarray(w_q, np.float32).T).astype(BF),
        "wk_t": np.ascontiguousarray(np.asarray(w_k, np.float32).T).astype(BF),
        "wv_t": np.ascontiguousarray(np.asarray(w_v, np.float32).T).astype(BF),
        "wo_t": np.ascontiguousarray(np.asarray(w_o, np.float32).T).astype(BF),
        "bq_r": np.ascontiguousarray(
            np.asarray(b_q, np.float32).reshape(ET, P).T),
        "bk_r": np.ascontiguousarray(
            np.asarray(b_k, np.float32).reshape(ET, P).T),
        "bvb": np.ascontiguousarray(
            np.tile(np.asarray(b_v, np.float32)[None, :], (P, 1))).astype(BF),
        "bob": np.ascontiguousarray(
            np.tile(np.asarray(b_o, np.float32)[None, :], (P, 1))).astype(BF),
    }
    q = np.asarray(query, np.float32)
    k = np.asarray(key_, np.float32)
    v = np.asarray(value, np.float32)
    in_maps = []
    for b in range(B):
        m = dict(shared)
        m["xq_t"] = np.ascontiguousarray(q[b].T).astype(BF)
        m["xk_t"] = np.ascontiguousarray(k[b].T).astype(BF)
        m["xv_t"] = np.ascontiguousarray(v[b].T).astype(BF)
        in_maps.append(m)
    return in_maps


def run(in_maps, repeat: int = 1):
    nc = get_nc(repeat)
    res = run_bass_kernel_spmd(nc, in_maps, list(range(N_CORES)))
    return np.stack([np.asarray(res.results[i]["out"], np.float32)
                     for i in range(B)])


def kernel(query, key_, value, w_q, b_q, w_k, b_k, w_v, b_v, w_o, b_o):
    in_maps = make_in_maps(query, key_, value, w_q, b_q, w_k, b_k,
                           w_v, b_v, w_o, b_o)
    return run(in_maps, repeat=1)


if __name__ == "__main__":
    rng = np.random.default_rng(0)
    sc = 1.0 / np.sqrt(D)
    inputs = dict(
        query=rng.standard_normal((B, S, D), dtype=np.float32),
        key_=rng.standard_normal((B, S, D), dtype=np.float32),
        value=rng.standard_normal((B, S, D), dtype=np.float32),
        w_q=rng.standard_normal((D, D), dtype=np.float32) * sc,
        b_q=np.zeros(D, np.float32),
        w_k=rng.standard_normal((D, D), dtype=np.float32) * sc,
        b_k=np.zeros(D, np.float32),
        w_v=rng.standard_normal((D, D), dtype=np.float32) * sc,
        b_v=np.zeros(D, np.float32),
        w_o=rng.standard_normal((D, D), dtype=np.float32) * sc,
        b_o=np.zeros(D, np.float32),
    )
    out = kernel(**inputs)
    print("out", out.shape, out.dtype, float(np.abs(out).max()))
